# revision 18
# baseline (speedup 1.0000x reference)
"""Multi-head attention forward (B=2, S=2048, E=1024, H=16, D=64) on 8 TRN2
NeuronCores, tensor-parallel across heads (2 heads/core).

Per core: QKV^T projection with X^T streamed as the moving operand, attention
computed in the S^T/attn^T orientation (softmax denominator obtained by
appending a ones column to V in the PV matmul), out-projection of the core's
128 embed dims giving a partial [4096, 1024] output. Host sums the 8 partials
and adds the output bias.
"""

import os
from contextlib import ExitStack

import numpy as np

import concourse.bass as bass
import concourse.mybir as mybir
import concourse.tile as tile
from concourse import bacc
from concourse.masks import make_identity

# ---- problem constants (hardcoded per contract) ----
B, S, E, H, D = 2, 2048, 1024, 16, 64
P = 128                      # partitions
R = B * S                    # 4096 flattened rows
KO = E // P                  # 8 contraction chunks over E
NKC = S // P                 # 16 key chunks per sequence
HC = 2                       # heads per core
NCORES = 8
RB = 512                     # row block for the QKV projection

# matmul-input mode: 'bf16' (cast inputs to bf16), 'f32r' (fp32 data, fast
# float32r matmuls), 'f32' (exact fp32, 4x slower PE)
MM_MODE = os.environ.get("MHA_MM_MODE", "f32r")

FP32 = mybir.dt.float32
EXP = mybir.ActivationFunctionType.Exp
IDENT = mybir.ActivationFunctionType.Identity


def _mode_params(mm_mode):
    if mm_mode == "bf16":
        return mybir.dt.bfloat16, 512
    elif mm_mode == "f32r":
        return mybir.dt.float32r, 512
    elif mm_mode == "f32":
        return FP32, 256
    raise ValueError(mm_mode)


def build_kernel(tc, xt, wqkv, bqkv, wout, y, sdt, QB, mm_mode, ctx):
    nc = tc.nc
    NQB = S // QB

    # float32r can only be produced by rounding-capable engine ops (ACT/DVE
    # outputs) or DMA of host-pre-rounded data; memset/affine_select cannot.
    # The transpose path therefore stays plain fp32 in f32r mode.
    vdt = FP32 if sdt == mybir.dt.float32r else sdt

    def mm(ap):
        return ap

    const = ctx.enter_context(tc.tile_pool(name="const", bufs=1))
    ps = ctx.enter_context(tc.tile_pool(name="ps", bufs=8, space="PSUM"))

    wq_sb = const.tile([P, KO, 3 * P], sdt)
    nc.sync.dma_start(wq_sb, wqkv.rearrange("(ko p) m -> p ko m", p=P))
    bq_sb = const.tile([P, 3], FP32)
    nc.sync.dma_start(bq_sb, bqkv.rearrange("(m p) -> p m", p=P))
    wo_sb = const.tile([P, E], sdt)
    nc.sync.dma_start(wo_sb, wout)
    ident = const.tile([P, P], vdt)
    make_identity(nc, ident)

    qt = const.tile([P, B, S], sdt)       # Q^T  [2h*64, b, s]
    kt = const.tile([P, B, S], sdt)       # K^T
    vt = const.tile([P, B, S], vdt)       # V^T
    v1 = const.tile([P, B, HC, NKC, D + 1], sdt)  # V natural + ones col
    attnT = const.tile([P, B, S], sdt)    # unnormalized-then-normalized attn^T

    ones_col = const.tile([P, 1], FP32)
    nc.vector.memset(ones_col, 1.0)
    nc.vector.tensor_copy(v1[:, :, :, :, D:],
                          ones_col.to_broadcast((P, B, HC, NKC, 1)))

    # ---- phase 1: QKV^T projection ----
    xt_r = xt.rearrange("(ko p) r -> p ko r", p=P)
    with tc.tile_pool(name="xtp", bufs=2) as xt_pool:
        for rb in range(R // RB):
            xt_t = xt_pool.tile([P, KO, RB], sdt, tag="xt")
            nc.sync.dma_start(xt_t, xt_r[:, :, rb * RB:(rb + 1) * RB])
            b, col = divmod(rb * RB, S)
            for m, dest in enumerate((qt, kt, vt)):
                pst = ps.tile([P, RB], FP32, tag="ps", name=f"ps_qkv_{rb}_{m}")
                for ko in range(KO):
                    nc.tensor.matmul(
                        pst, mm(wq_sb[:, ko, m * P:(m + 1) * P]),
                        mm(xt_t[:, ko, :]),
                        start=(ko == 0), stop=(ko == KO - 1))
                nc.scalar.activation(dest[:, b, col:col + RB], pst, IDENT,
                                     bias=bq_sb[:, m:m + 1])

    # ---- phase 2: V natural via PE transpose ----
    for b in range(B):
        for kc in range(NKC):
            for h in range(HC):
                pst = ps.tile([P, D], vdt, tag="ps", name=f"ps_tr_{b}_{kc}_{h}")
                nc.tensor.transpose(
                    pst, vt[h * D:(h + 1) * D, b, kc * P:(kc + 1) * P],
                    ident[h * D:(h + 1) * D, h * D:(h + 1) * D])
                nc.vector.tensor_copy(v1[:, b, h, kc, :D], pst)

    # ---- phase 3+4: attention + out-projection ----
    exps_pool = ctx.enter_context(tc.tile_pool(name="exps", bufs=2))
    rc_pool = ctx.enter_context(tc.tile_pool(name="rc", bufs=2))
    bc_pool = ctx.enter_context(tc.tile_pool(name="bc", bufs=2))
    y_pool = ctx.enter_context(tc.tile_pool(name="yp", bufs=3))
    for b in range(B):
        for qb in range(NQB):
            ess = []
            for h in range(HC):
                ess.append(exps_pool.tile([P, NKC, QB], sdt, tag="es",
                                          name=f"es_{b}_{qb}_{h}"))
            # scores for both heads issued per-kc back to back: the two
            # matmuls use disjoint 64-row groups of the PE array (head 0 in
            # rows 0:63, head 1 in rows 64:127) and run concurrently.
            for kc in range(NKC):
                for h in range(HC):
                    pst = ps.tile([P, QB], FP32, tag="ps",
                                  name=f"ps_sc_{b}_{qb}_{h}_{kc}")
                    nc.tensor.matmul(
                        pst,
                        mm(kt[h * D:(h + 1) * D, b, kc * P:(kc + 1) * P]),
                        mm(qt[h * D:(h + 1) * D, b, qb * QB:(qb + 1) * QB]),
                        start=True, stop=True)
                    nc.scalar.activation(ess[h][:, kc, :], pst, EXP, scale=0.125)
            for h in range(HC):
                es = ess[h]
                pa = ps.tile([P, QB], FP32, tag="ps", name=f"ps_at_{b}_{qb}_{h}")
                for kc in range(NKC):
                    nc.tensor.matmul(
                        pa[:D + 1, :], mm(v1[:, b, h, kc, :]), mm(es[:, kc, :]),
                        start=(kc == 0), stop=(kc == NKC - 1))
                rc = rc_pool.tile([1, QB], FP32, tag="rc", name=f"rc_{b}_{qb}_{h}")
                nc.vector.reciprocal(rc, pa[D:D + 1, :])
                bc = bc_pool.tile([D, QB], FP32, tag="bc", name=f"bc_{b}_{qb}_{h}")
                nc.gpsimd.partition_broadcast(bc, rc)
                nc.vector.tensor_tensor(
                    attnT[h * D:(h + 1) * D, b, qb * QB:(qb + 1) * QB],
                    pa[:D, :], bc, mybir.AluOpType.mult)
            for qc in range(QB // P):
                q0 = qb * QB + qc * P
                yt = y_pool.tile([P, E], FP32, tag="yt", name=f"yt_{b}_{qb}_{qc}")
                for nh in range(2):
                    pst = ps.tile([P, 512], FP32, tag="ps",
                                  name=f"ps_y_{b}_{qb}_{qc}_{nh}")
                    nc.tensor.matmul(
                        pst, mm(attnT[:, b, q0:q0 + P]),
                        mm(wo_sb[:, nh * 512:(nh + 1) * 512]),
                        start=True, stop=True)
                    nc.vector.tensor_copy(yt[:, nh * 512:(nh + 1) * 512], pst)
                nc.sync.dma_start(y[b * S + q0: b * S + q0 + P, :], yt)


def build_nc(mm_mode=MM_MODE, reps=1):
    sdt, QB = _mode_params(mm_mode)
    nc = bacc.Bacc("TRN2", target_bir_lowering=False, debug=False)
    xt = nc.dram_tensor("xt", [E, R], sdt, kind="ExternalInput").ap()
    wqkv = nc.dram_tensor("wqkv", [E, 3 * P], sdt, kind="ExternalInput").ap()
    bqkv = nc.dram_tensor("bqkv", [3 * P], FP32, kind="ExternalInput").ap()
    wout = nc.dram_tensor("wout", [P, E], sdt, kind="ExternalInput").ap()
    y = nc.dram_tensor("y", [R, E], FP32, kind="ExternalOutput").ap()
    with tile.TileContext(nc) as tc:
        for _ in range(reps):
            with ExitStack() as ctx:
                build_kernel(tc, xt, wqkv, bqkv, wout, y, sdt, QB, mm_mode, ctx)
    nc.compile()
    return nc


def _round_f32r(x):
    """Round fp32 to the fp32r grid (11 explicit mantissa bits) the way the
    hardware expects matmul operands: add-half then truncate the low 12 bits."""
    bits = np.ascontiguousarray(x, np.float32).view(np.uint32)
    return (((bits + np.uint32(0x800)) & np.uint32(0xFFFFF000))
            .view(np.float32))


def shard_inputs(input_tensor, qkv_w, qkv_b, out_w, mm_mode=MM_MODE):
    """Build the 8 per-core input maps (numpy, host-side)."""
    sdt, _ = _mode_params(mm_mode)
    np_sdt = mybir.dt.np(sdt)

    def prep(a):
        a = np.ascontiguousarray(a).astype(np_sdt)
        return _round_f32r(a) if mm_mode == "f32r" else a

    X = np.asarray(input_tensor, np.float32).reshape(R, E)
    XT = prep(X.T)
    qkv_w = np.asarray(qkv_w, np.float32)
    qkv_b = np.asarray(qkv_b, np.float32)
    out_w = np.asarray(out_w, np.float32)
    in_maps = []
    for c in range(NCORES):
        sl = slice(c * P, (c + 1) * P)
        wq = np.concatenate(
            [qkv_w[:, sl], qkv_w[:, E + c * P:E + (c + 1) * P],
             qkv_w[:, 2 * E + c * P:2 * E + (c + 1) * P]], axis=1)
        bq = np.concatenate(
            [qkv_b[sl], qkv_b[E + c * P:E + (c + 1) * P],
             qkv_b[2 * E + c * P:2 * E + (c + 1) * P]])
        in_maps.append({
            "xt": XT,
            "wqkv": prep(wq),
            "bqkv": np.ascontiguousarray(bq),
            "wout": prep(out_w[sl, :]),
        })
    return in_maps


_NC_CACHE = {}


def _get_nc(mm_mode):
    if mm_mode not in _NC_CACHE:
        _NC_CACHE[mm_mode] = build_nc(mm_mode)
    return _NC_CACHE[mm_mode]


LAST_RESULT = None


def kernel(input_tensor, qkv_w, qkv_b, out_w, out_b):
    global LAST_RESULT
    from concourse import bass_utils
    nc = _get_nc(MM_MODE)
    in_maps = shard_inputs(input_tensor, qkv_w, qkv_b, out_w, MM_MODE)
    res = bass_utils.run_bass_kernel_spmd(
        nc, in_maps, core_ids=list(range(NCORES)),
        trace=bool(int(os.environ.get("MHA_TRACE", "0"))))
    LAST_RESULT = res
    out = np.zeros((R, E), np.float32)
    for r in res.results:
        out += r["y"]
    out += np.asarray(out_b, np.float32)
    return out.reshape(B, S, E)


def core_partial_ref(input_tensor, qkv_w, qkv_b, out_w, c):
    """Exact fp32 numpy reference for core c's partial output (for testing)."""
    X = np.asarray(input_tensor, np.float32).reshape(R, E)
    sl = slice(c * P, (c + 1) * P)
    out = np.zeros((R, E), np.float32)
    for b in range(B):
        rows = slice(b * S, (b + 1) * S)
        for hl in range(HC):
            h = c * HC + hl
            q = X[rows] @ qkv_w[:, h * D:(h + 1) * D] + qkv_b[h * D:(h + 1) * D]
            k = X[rows] @ qkv_w[:, E + h * D:E + (h + 1) * D] + qkv_b[E + h * D:E + (h + 1) * D]
            v = X[rows] @ qkv_w[:, 2 * E + h * D:2 * E + (h + 1) * D] + qkv_b[2 * E + h * D:2 * E + (h + 1) * D]
            s = (q @ k.T) / np.sqrt(np.float32(D))
            p = np.exp(s - s.max(axis=1, keepdims=True))
            p /= p.sum(axis=1, keepdims=True)
            a = p @ v
            out[rows] += a @ out_w[h * D:(h + 1) * D, :]
    return out


# revision 29
# speedup vs baseline: 5.7477x; 5.7477x over previous
"""Multi-head attention forward (B=2, S=2048, E=1024, H=16, D=64) on 8 TRN2
NeuronCores, tensor-parallel across heads (2 heads/core).

Per core: QKV^T projection with X^T streamed as the moving operand, attention
computed in the S^T/attn^T orientation (softmax denominator obtained by
appending a ones column to V in the PV matmul), out-projection of the core's
128 embed dims giving a partial [4096, 1024] output. Host sums the 8 partials
and adds the output bias.
"""

import os
from contextlib import ExitStack

import numpy as np

import concourse.bass as bass
import concourse.mybir as mybir
import concourse.tile as tile
from concourse import bacc
from concourse.masks import make_identity

# ---- problem constants (hardcoded per contract) ----
B, S, E, H, D = 2, 2048, 1024, 16, 64
P = 128                      # partitions
R = B * S                    # 4096 flattened rows
KO = E // P                  # 8 contraction chunks over E
NKC = S // P                 # 16 key chunks per sequence
HC = 2                       # heads per core
NCORES = 8
RB = 512                     # row block for the QKV projection

# matmul-input mode: 'bf16' (cast inputs to bf16), 'f32r' (fp32 data, fast
# float32r matmuls), 'f32' (exact fp32, 4x slower PE)
MM_MODE = os.environ.get("MHA_MM_MODE", "f32r")
QB_OVERRIDE = int(os.environ.get("MHA_QB", "0"))        # 0 = mode default
PACK_SCORES = bool(int(os.environ.get("MHA_PACK", "1")))
ES_BUFS = int(os.environ.get("MHA_ES_BUFS", "8"))
KCG = int(os.environ.get("MHA_KCG", "2"))               # kc per exp group
SC_BUFS = int(os.environ.get("MHA_SC_BUFS", "2"))

FP32 = mybir.dt.float32
EXP = mybir.ActivationFunctionType.Exp
IDENT = mybir.ActivationFunctionType.Identity


def _mode_params(mm_mode):
    if mm_mode == "bf16":
        dt, qb = mybir.dt.bfloat16, 512
    elif mm_mode == "f32r":
        dt, qb = mybir.dt.float32r, 512
    elif mm_mode == "f32":
        dt, qb = FP32, 256
    else:
        raise ValueError(mm_mode)
    return dt, (QB_OVERRIDE or qb)


def build_kernel(tc, xt, wqkv, bqkv, wout, y, sdt, QB, mm_mode, ctx):
    nc = tc.nc
    NQB = S // QB
    NRB = S // RB            # row blocks per batch

    # float32r can only be produced by rounding-capable engine ops (ACT/DVE
    # outputs) or DMA of host-pre-rounded data; memset/affine_select cannot.
    # The transpose path therefore stays plain fp32 in f32r mode.
    vdt = FP32 if sdt == mybir.dt.float32r else sdt

    def mm(ap):
        return ap

    const = ctx.enter_context(tc.tile_pool(name="const", bufs=1))
    # PSUM budget: 8 banks = scores 2x2 (KCG banks per tile) + qkv/transpose
    # 2x1 + pa/out-proj shared 2x1.
    ps_sc = ctx.enter_context(tc.tile_pool(name="ps_sc", bufs=SC_BUFS,
                                           space="PSUM"))
    ps_q = ctx.enter_context(tc.tile_pool(name="ps_q", bufs=2, space="PSUM"))
    ps_pa = ctx.enter_context(tc.tile_pool(name="ps_pa", bufs=2, space="PSUM"))

    wq_sb = const.tile([P, KO, 3 * P], sdt)
    wq_r = wqkv.rearrange("(ko p) m -> p ko m", p=P)
    for ko in range(KO):
        nc.sync.dma_start(wq_sb[:, ko, :], wq_r[:, ko, :])
    bq_sb = const.tile([P, 3], FP32)
    nc.sync.dma_start(bq_sb, bqkv.rearrange("(m p) -> p m", p=P))
    wo_sb = const.tile([P, E], sdt)
    nc.sync.dma_start(wo_sb, wout)
    ident = const.tile([P, P], vdt)
    make_identity(nc, ident)

    qt = const.tile([P, B, S], sdt)       # Q^T  [2h*64, b, s]
    kt = const.tile([P, B, S], sdt)       # K^T
    vt = const.tile([P, B, S], vdt)       # V^T
    v1 = const.tile([P, B, HC, NKC, D + 1], sdt)  # V natural + ones col
    attnT = const.tile([P, B, S], sdt)    # unnormalized-then-normalized attn^T

    ones_col = const.tile([P, 1], FP32)
    nc.vector.memset(ones_col, 1.0)
    nc.vector.tensor_copy(v1[:, :, :, :, D:],
                          ones_col.to_broadcast((P, B, HC, NKC, 1)))

    xt_pool = ctx.enter_context(tc.tile_pool(name="xtp", bufs=2))
    exps_pool = ctx.enter_context(tc.tile_pool(name="exps", bufs=ES_BUFS))
    rc_pool = ctx.enter_context(tc.tile_pool(name="rc", bufs=2))
    bc_pool = ctx.enter_context(tc.tile_pool(name="bc", bufs=2))
    y_pool = ctx.enter_context(tc.tile_pool(name="yp", bufs=3))

    xt_r = xt.rearrange("(ko p) r -> p ko r", p=P)
    NG = NKC // KCG

    for b in range(B):
        # ---- QKV^T projection for batch b, V transposes fused in ----
        for rbi in range(NRB):
            rb = b * NRB + rbi
            col = rbi * RB
            xt_t = xt_pool.tile([P, KO, RB], sdt, tag="xt")
            for ko in range(KO):
                nc.sync.dma_start(xt_t[:, ko, :],
                                  xt_r[:, ko, rb * RB:(rb + 1) * RB])
            for m, dest in enumerate((qt, kt, vt)):
                pst = ps_q.tile([P, RB], FP32, tag="pq", name=f"ps_qkv_{rb}_{m}")
                for ko in range(KO):
                    nc.tensor.matmul(
                        pst, mm(wq_sb[:, ko, m * P:(m + 1) * P]),
                        mm(xt_t[:, ko, :]),
                        start=(ko == 0), stop=(ko == KO - 1))
                nc.vector.tensor_scalar_add(dest[:, b, col:col + RB], pst,
                                            bq_sb[:, m:m + 1])
            # V natural via PE transpose for this row block's kc chunks
            for kci in range(RB // P):
                kc = (col // P) + kci
                for h in range(HC):
                    pst = ps_q.tile([P, D], vdt, tag="pq",
                                    name=f"ps_tr_{b}_{kc}_{h}")
                    nc.tensor.transpose(
                        pst, vt[h * D:(h + 1) * D, b, kc * P:(kc + 1) * P],
                        ident[h * D:(h + 1) * D, h * D:(h + 1) * D])
                    nc.vector.tensor_copy(v1[:, b, h, kc, :D], pst)

        # ---- attention + out-projection for batch b ----
        # kc-groups of KCG: exp (ScalarE) of one group overlaps scores/PV
        # matmuls of neighbouring groups on the PE.
        for qb in range(NQB):
            pa = {}
            for h in range(HC):
                pa[h] = ps_pa.tile([P, QB], FP32, tag="pa",
                                   name=f"pa_{b}_{qb}_{h}")
            for g in range(NG):
                est = {}
                pst = {}
                for h in range(HC):
                    est[h] = exps_pool.tile([P, KCG, QB], sdt, tag="es",
                                            name=f"es_{b}_{qb}_{g}_{h}")
                    pst[h] = ps_sc.tile([P, KCG, QB], FP32, tag="sc",
                                        name=f"ps_sc_{b}_{qb}_{g}_{h}")
                # scores, heads interleaved per kc (disjoint 64-row PE
                # groups run concurrently); one multi-bank exp per head.
                if PACK_SCORES:
                    jh = [(j, h) for j in range(KCG) for h in range(HC)]
                else:
                    jh = [(j, h) for h in range(HC) for j in range(KCG)]
                for j, h in jh:
                    kc = g * KCG + j
                    nc.tensor.matmul(
                        pst[h][:, j, :],
                        mm(kt[h * D:(h + 1) * D, b, kc * P:(kc + 1) * P]),
                        mm(qt[h * D:(h + 1) * D, b, qb * QB:(qb + 1) * QB]),
                        start=True, stop=True)
                for h in range(HC):
                    nc.scalar.activation(est[h], pst[h], EXP, scale=0.125)
                for h in range(HC):
                    for j in range(KCG):
                        kc = g * KCG + j
                        nc.tensor.matmul(
                            pa[h][:D + 1, :], mm(v1[:, b, h, kc, :]),
                            mm(est[h][:, j, :]),
                            start=(kc == 0), stop=(kc == NKC - 1),
                            skip_group_check=True)
            for h in range(HC):
                rc = rc_pool.tile([1, QB], FP32, tag="rc", name=f"rc_{b}_{qb}_{h}")
                nc.vector.reciprocal(rc, pa[h][D:D + 1, :])
                bc = bc_pool.tile([D, QB], FP32, tag="bc", name=f"bc_{b}_{qb}_{h}")
                nc.gpsimd.partition_broadcast(bc, rc)
                nc.vector.tensor_tensor(
                    attnT[h * D:(h + 1) * D, b, qb * QB:(qb + 1) * QB],
                    pa[h][:D, :], bc, mybir.AluOpType.mult)
            for qc in range(QB // P):
                q0 = qb * QB + qc * P
                yt = y_pool.tile([P, E], FP32, tag="yt", name=f"yt_{b}_{qb}_{qc}")
                for nh in range(2):
                    pst = ps_pa.tile([P, 512], FP32, tag="pa",
                                     name=f"ps_y_{b}_{qb}_{qc}_{nh}")
                    nc.tensor.matmul(
                        pst, mm(attnT[:, b, q0:q0 + P]),
                        mm(wo_sb[:, nh * 512:(nh + 1) * 512]),
                        start=True, stop=True)
                    nc.vector.tensor_copy(yt[:, nh * 512:(nh + 1) * 512], pst)
                nc.sync.dma_start(y[b * S + q0: b * S + q0 + P, :], yt)


def build_nc(mm_mode=MM_MODE, reps=1):
    sdt, QB = _mode_params(mm_mode)
    nc = bacc.Bacc("TRN2", target_bir_lowering=False, debug=False)
    xt = nc.dram_tensor("xt", [E, R], sdt, kind="ExternalInput").ap()
    wqkv = nc.dram_tensor("wqkv", [E, 3 * P], sdt, kind="ExternalInput").ap()
    bqkv = nc.dram_tensor("bqkv", [3 * P], FP32, kind="ExternalInput").ap()
    wout = nc.dram_tensor("wout", [P, E], sdt, kind="ExternalInput").ap()
    y = nc.dram_tensor("y", [R, E], FP32, kind="ExternalOutput").ap()
    with tile.TileContext(nc) as tc:
        for _ in range(reps):
            with ExitStack() as ctx:
                build_kernel(tc, xt, wqkv, bqkv, wout, y, sdt, QB, mm_mode, ctx)
    nc.compile()
    return nc


def _round_f32r(x):
    """Round fp32 to the fp32r grid (11 explicit mantissa bits) the way the
    hardware expects matmul operands: add-half then truncate the low 12 bits."""
    bits = np.ascontiguousarray(x, np.float32).view(np.uint32)
    return (((bits + np.uint32(0x800)) & np.uint32(0xFFFFF000))
            .view(np.float32))


def shard_inputs(input_tensor, qkv_w, qkv_b, out_w, mm_mode=MM_MODE):
    """Build the 8 per-core input maps (numpy, host-side)."""
    sdt, _ = _mode_params(mm_mode)
    np_sdt = mybir.dt.np(sdt)

    def prep(a):
        a = np.ascontiguousarray(a).astype(np_sdt)
        return _round_f32r(a) if mm_mode == "f32r" else a

    X = np.asarray(input_tensor, np.float32).reshape(R, E)
    XT = prep(X.T)
    qkv_w = np.asarray(qkv_w, np.float32)
    qkv_b = np.asarray(qkv_b, np.float32)
    out_w = np.asarray(out_w, np.float32)
    in_maps = []
    for c in range(NCORES):
        sl = slice(c * P, (c + 1) * P)
        wq = np.concatenate(
            [qkv_w[:, sl], qkv_w[:, E + c * P:E + (c + 1) * P],
             qkv_w[:, 2 * E + c * P:2 * E + (c + 1) * P]], axis=1)
        bq = np.concatenate(
            [qkv_b[sl], qkv_b[E + c * P:E + (c + 1) * P],
             qkv_b[2 * E + c * P:2 * E + (c + 1) * P]])
        in_maps.append({
            "xt": XT,
            "wqkv": prep(wq),
            "bqkv": np.ascontiguousarray(bq),
            "wout": prep(out_w[sl, :]),
        })
    return in_maps


_NC_CACHE = {}


def _get_nc(mm_mode):
    if mm_mode not in _NC_CACHE:
        _NC_CACHE[mm_mode] = build_nc(mm_mode)
    return _NC_CACHE[mm_mode]


LAST_RESULT = None


def kernel(input_tensor, qkv_w, qkv_b, out_w, out_b):
    global LAST_RESULT
    from concourse import bass_utils
    nc = _get_nc(MM_MODE)
    in_maps = shard_inputs(input_tensor, qkv_w, qkv_b, out_w, MM_MODE)
    res = bass_utils.run_bass_kernel_spmd(
        nc, in_maps, core_ids=list(range(NCORES)),
        trace=bool(int(os.environ.get("MHA_TRACE", "0"))))
    LAST_RESULT = res
    out = np.zeros((R, E), np.float32)
    for r in res.results:
        out += r["y"]
    out += np.asarray(out_b, np.float32)
    return out.reshape(B, S, E)


def core_partial_ref(input_tensor, qkv_w, qkv_b, out_w, c):
    """Exact fp32 numpy reference for core c's partial output (for testing)."""
    X = np.asarray(input_tensor, np.float32).reshape(R, E)
    out = np.zeros((R, E), np.float32)
    for b in range(B):
        rows = slice(b * S, (b + 1) * S)
        for hl in range(HC):
            h = c * HC + hl
            q = X[rows] @ qkv_w[:, h * D:(h + 1) * D] + qkv_b[h * D:(h + 1) * D]
            k = X[rows] @ qkv_w[:, E + h * D:E + (h + 1) * D] + qkv_b[E + h * D:E + (h + 1) * D]
            v = X[rows] @ qkv_w[:, 2 * E + h * D:2 * E + (h + 1) * D] + qkv_b[2 * E + h * D:2 * E + (h + 1) * D]
            s = (q @ k.T) / np.sqrt(np.float32(D))
            p = np.exp(s - s.max(axis=1, keepdims=True))
            p /= p.sum(axis=1, keepdims=True)
            a = p @ v
            out[rows] += a @ out_w[h * D:(h + 1) * D, :]
    return out

